# revision 17
# baseline (speedup 1.0000x reference)
"""Hierarchical-softmax loss kernel for Trainium2 (8 NeuronCores).

Strategy
--------
Data-parallel over the n_ex dimension. Examples are globally sorted by
path length (descending) and dealt round-robin to the 8 cores so every
core sees a near-identical length profile; within a core, examples form
8 partition-tiles of 128 whose step loops run only to the tile max
length (~55% of the dense 24-step bound).

W is pre-scaled (x512) and stored in DRAM as fp8-e4m3, quartering the
HBM gather traffic vs f32. Per (tile, step-chunk) ONE multi-column
indirect DMA gathers up to 8 W rows per partition, casting fp8 -> bf16
in flight (994ns SWDGE fixed cost paid once per ~1024 rows instead of
once per 128). Per-step dot products are split between the DVE (fused
tensor_tensor_reduce) and DVE-mult + ACT-accum pairs at a ratio that
balances both engines. A small affine (dot * -code/512 + mask_bias)
followed by a stable softplus chain with ScalarE accumulation produces
per-partition partial sums; the final reduction over 8192 partials
happens on host.
"""

import os
import sys

import numpy as np

for _p in ("/opt/trn_rl_repo", "/root/.axon_site/_ro/trn_rl_repo"):
    if os.path.isdir(_p) and _p not in sys.path:
        sys.path.append(_p)

V = 50257
N_DEC = V - 1
D = 1024
N_EX = 8192
MAX_LEN = 24
N_CORES = 8
P = 128
N_TILES = N_EX // (N_CORES * P)  # 8 example-tiles of 128 per core
MASK_BIAS = -30.0                # softplus(-30) ~ 9e-14 == masked-out step
W_SCALE = 512.0                  # fp8-e4m3 pre-scale to dodge subnormals
CHUNK = 8                        # steps gathered per indirect DMA
TTR_MOD = 4                      # of every 4 steps, this many ...
TTR_CNT = 1                      # ... reduce on DVE instead of ACT

_prog_cache: dict = {}


def _patch_tail_drain(tile, mybir, bass_rust):
    """The pinned walrus encodes only a limited number of sync-waits per CTRL
    instruction, but Tile's kernel-tail Drain carries one wait per active
    processor lane. Spread the extra waits over single-wait NOPs."""
    if getattr(tile.TileContext._drain_and_barrier, "_split_waits", False):
        return

    def _drain_and_barrier(self, tick_clock, wait_clock):
        nc = self.nc
        drain_inst = nc.sync.drain()
        wait_clock.add_sem_waits(
            drain_inst.ins, bass_rust.ScopedClock({None: tick_clock.global_clock})
        )
        si = drain_inst.ins.sync_info
        waits = list(si.on_wait or [])
        if len(waits) > 1:
            si.on_wait = waits[:1]
            for w in waits[1:]:
                nop = nc.sync.nop(nofuse=True)
                nop.ins.sync_info = mybir.SyncInfo(on_wait=[w], on_update=[])
        nc.all_engine_barrier()
        popped = nc._tile_sem_poison_stack.pop()
        assert popped is self._sem_poison
        nc.clear_and_free_semaphores(list(self.sems.allocated().values()))
        nc.all_engine_barrier()

    _drain_and_barrier._split_waits = True
    tile.TileContext._drain_and_barrier = _drain_and_barrier


def _split_multiwait_instructions(nc, mybir, maxw=1):
    """Hoist extra sem-waits from any instruction onto single-wait NOPs placed
    immediately before it on the same engine (same aggregate wait semantics)."""
    f = nc.m.functions[0]
    tail = nc.cur_bb.bb
    blocks = list(f.blocks)
    if not any(b.name == tail.name for b in blocks):
        blocks.append(tail)
    for blk in blocks:
        snapshot = list(blk.instructions)
        heavy = [
            i for i in snapshot
            if i.sync_info and i.sync_info.on_wait and len(i.sync_info.on_wait) > maxw
        ]
        if not heavy:
            continue
        pre_len = len(tail.instructions)
        n_created = 0
        new_list = []
        for inst in snapshot:
            si = inst.sync_info
            if si and si.on_wait and len(si.on_wait) > maxw:
                waits = list(si.on_wait)
                extra, keep = waits[:-maxw], waits[-maxw:]
                si.on_wait = keep
                for w in extra:
                    nop = nc.engines[inst.engine].nop(nofuse=True)
                    nop.ins.sync_info = mybir.SyncInfo(on_wait=[w], on_update=[])
                    new_list.append(nop.ins)
                    n_created += 1
            new_list.append(inst)
        # builder appended the fresh NOPs to the tail block; strip them there
        t = list(tail.instructions)
        assert len(t) == pre_len + n_created
        if blk.name == tail.name:
            blk.instructions = new_list
        else:
            tail.instructions = t[:pre_len]
            blk.instructions = new_list


def _build_program(lmax: tuple):
    from concourse import bass, mybir
    import concourse.tile as tile
    import bass_rust

    _patch_tail_drain(tile, mybir, bass_rust)

    ltot = int(sum(lmax))
    nc = bass.Bass("TRN2", target_bir_lowering=False)
    f32 = mybir.dt.float32
    bf16 = mybir.dt.bfloat16
    fp8 = mybir.dt.float8e4

    xs = nc.declare_dram_parameter("xs", [N_TILES * P, D], bf16, isOutput=False)
    W = nc.declare_dram_parameter("W", [N_DEC, D], fp8, isOutput=False)
    gidx = nc.declare_dram_parameter("gidx", [P, ltot], mybir.dt.int32, isOutput=False)
    nsc = nc.declare_dram_parameter("nsc", [P, N_TILES * MAX_LEN], f32, isOutput=False)
    mbs = nc.declare_dram_parameter("mbs", [P, N_TILES * MAX_LEN], f32, isOutput=False)
    out = nc.declare_dram_parameter("out", [P, 1], f32, isOutput=True)

    with tile.TileContext(nc) as tc:
        with (
            tc.tile_pool(name="xpool", bufs=1) as xpool,
            tc.tile_pool(name="gpool", bufs=4) as gpool,
            tc.tile_pool(name="meta", bufs=1) as meta,
            tc.tile_pool(name="scratch", bufs=6) as scratch,
            tc.tile_pool(name="dpool", bufs=2) as dpool,
            tc.tile_pool(name="outp", bufs=1) as outp,
        ):
            gidx_t = meta.tile([P, ltot], mybir.dt.int32, tag="gidx")
            nsc_t = meta.tile([P, N_TILES * MAX_LEN], f32, tag="nsc")
            mbs_t = meta.tile([P, N_TILES * MAX_LEN], f32, tag="mbs")
            nc.sync.dma_start(out=gidx_t[:], in_=gidx[:, :])
            nc.sync.dma_start(out=nsc_t[:], in_=nsc[:, :])
            nc.sync.dma_start(out=mbs_t[:], in_=mbs[:, :])

            parts = outp.tile([P, 1], f32, tag="parts")
            pa = outp.tile([P, 1], f32, tag="pa")
            pb = outp.tile([P, 1], f32, tag="pb")

            xt = []
            for k in range(N_TILES):
                xk = xpool.tile([P, D], bf16, tag=f"x{k}")
                nc.sync.dma_start(out=xk[:], in_=xs[k * P : (k + 1) * P, :])
                xt.append(xk)

            # one dots buffer for all tiles; padded columns stay 0 via memset
            dots = dpool.tile([P, N_TILES * MAX_LEN], f32, tag="dots")
            nc.vector.memset(dots[:], 0.0)

            ridx = 0
            it0 = 0
            for k in range(N_TILES):
                lm = int(lmax[k])
                for c0 in range(0, lm, CHUNK):
                    cw = min(CHUNK, lm - c0)
                    # gather cw rows per partition in ONE indirect DMA,
                    # casting fp8 -> bf16 in flight
                    g = gpool.tile([P, CHUNK * D], bf16, tag="g")
                    nc.gpsimd.indirect_dma_start(
                        out=g[:, : cw * D],
                        out_offset=None,
                        in_=W[:, :],
                        in_offset=bass.IndirectOffsetOnAxis(
                            ap=gidx_t[:, it0 + c0 : it0 + c0 + cw], axis=0
                        ),
                    )
                    # batched multiply (4 steps per DVE op, x broadcast), then
                    # a half-fold per batch, then FD-512 ACT accums per step
                    for j0 in range(0, cw, 4):
                        bw = min(4, cw - j0)
                        ps = scratch.tile([P, 4 * D], bf16, tag="ps")
                        nc.vector.tensor_tensor(
                            out=ps[:, : bw * D].rearrange("p (s d) -> p s d", s=bw),
                            in0=xt[k][:].unsqueeze(1).broadcast_to([P, bw, D]),
                            in1=g[:, j0 * D : (j0 + bw) * D].rearrange(
                                "p (s d) -> p s d", s=bw
                            ),
                            op=mybir.AluOpType.mult,
                        )
                        pf = scratch.tile([P, 4 * (D // 2)], bf16, tag="pf")
                        ps4 = ps[:, : bw * D].rearrange(
                            "p (s h d) -> p s h d", s=bw, h=2
                        )
                        nc.vector.tensor_tensor(
                            out=pf[:, : bw * (D // 2)].rearrange(
                                "p (s d) -> p s d", s=bw
                            ),
                            in0=ps4[:, :, 0, :],
                            in1=ps4[:, :, 1, :],
                            op=mybir.AluOpType.add,
                        )
                        for j in range(bw):
                            l0 = c0 + j0 + j
                            dcol = dots[:, k * MAX_LEN + l0 : k * MAX_LEN + l0 + 1]
                            ps2 = scratch.tile([P, D // 2], bf16, tag="ps2")
                            nc.scalar.activation(
                                out=ps2[:],
                                in_=pf[:, j * (D // 2) : (j + 1) * (D // 2)],
                                func=mybir.ActivationFunctionType.Copy,
                                accum_out=dcol,
                            )
                            ridx += 1
                it0 += lm

            # v = dot * (-code/W_SCALE) + mask_bias over all tiles at once
            nc.vector.tensor_tensor(
                out=dots[:], in0=dots[:], in1=nsc_t[:], op=mybir.AluOpType.mult
            )
            nc.vector.tensor_tensor(
                out=dots[:], in0=dots[:], in1=mbs_t[:], op=mybir.AluOpType.add
            )
            # stable softplus(v) = relu(v) + ln(1 + exp(-|v|))
            va = dpool.tile([P, N_TILES * MAX_LEN], f32, tag="va")
            nc.scalar.activation(
                out=va[:], in_=dots[:], func=mybir.ActivationFunctionType.Abs
            )
            ve = dpool.tile([P, N_TILES * MAX_LEN], f32, tag="ve")
            nc.scalar.activation(
                out=ve[:], in_=va[:],
                func=mybir.ActivationFunctionType.Exp, scale=-1.0,
            )
            vl = dpool.tile([P, N_TILES * MAX_LEN], f32, tag="vl")
            nc.scalar.activation(
                out=vl[:], in_=ve[:],
                func=mybir.ActivationFunctionType.Ln, bias=1.0,
                accum_out=pb[:, :],
            )
            vr = dpool.tile([P, N_TILES * MAX_LEN], f32, tag="vr")
            nc.scalar.activation(
                out=vr[:], in_=dots[:],
                func=mybir.ActivationFunctionType.Relu,
                accum_out=pa[:, :],
            )

            nc.vector.tensor_tensor(
                out=parts[:], in0=pa[:], in1=pb[:], op=mybir.AluOpType.add
            )
            nc.sync.dma_start(out=out[:, :], in_=parts[:])

    _split_multiwait_instructions(nc, mybir)
    return nc


def _prepare(x, W, t, paths, codes, lens):
    """Host-side index prep: length-sorted round-robin shard + per-core tables."""
    import ml_dtypes

    L = lens[t].astype(np.int64)                      # [N_EX]
    rank = np.argsort(-L, kind="stable")              # examples by length desc

    # slot s (0..1023) of core c takes example rank[s*8 + c]
    sel = rank.reshape(N_CORES * N_TILES * P // N_CORES, N_CORES)  # [1024, 8]
    # per-tile common max length (rank band head)
    lmax = tuple(int(L[rank[k * (N_CORES * P)]]) for k in range(N_TILES))
    ltot = int(sum(lmax))

    Wq = (W * W_SCALE).astype(ml_dtypes.float8_e4m3)  # [N_DEC, D] fp8

    in_maps = []
    for c in range(N_CORES):
        ex = sel[:, c]                                # [1024] example ids
        xs_c = np.ascontiguousarray(x[ex]).astype(ml_dtypes.bfloat16)
        t_c = t[ex]
        node_c = paths[t_c]                           # [1024, MAX_LEN] int32
        code_c = codes[t_c]                           # [1024, MAX_LEN] f32
        L_c = L[ex]                                   # [1024]

        gidx_c = np.zeros((P, ltot), dtype=np.int32)
        nsc_c = np.zeros((P, N_TILES * MAX_LEN), dtype=np.float32)
        mbs_c = np.full((P, N_TILES * MAX_LEN), MASK_BIAS, dtype=np.float32)
        it0 = 0
        for k in range(N_TILES):
            lm = lmax[k]
            rows = slice(k * P, (k + 1) * P)
            valid = np.arange(lm)[None, :] < L_c[rows][:, None]   # [P, lm]
            gidx_c[:, it0 : it0 + lm] = np.where(valid, node_c[rows, :lm], 0)
            nsc_c[:, k * MAX_LEN : k * MAX_LEN + lm] = np.where(
                valid, -code_c[rows, :lm] / W_SCALE, 0.0
            )
            mbs_c[:, k * MAX_LEN : k * MAX_LEN + lm] = np.where(valid, 0.0, MASK_BIAS)
            it0 += lm

        in_maps.append(
            {
                "xs": xs_c,
                "W": Wq,
                "gidx": gidx_c,
                "nsc": nsc_c,
                "mbs": mbs_c,
            }
        )
    return lmax, in_maps


def kernel(x, W, t, paths, codes, lens):
    from concourse import bass_utils

    lmax, in_maps = _prepare(
        np.asarray(x), np.asarray(W), np.asarray(t),
        np.asarray(paths), np.asarray(codes), np.asarray(lens),
    )
    nc = _prog_cache.get(lmax)
    if nc is None:
        nc = _build_program(lmax)
        _prog_cache[lmax] = nc

    res = bass_utils.run_bass_kernel_spmd(nc, in_maps, core_ids=list(range(N_CORES)))
    total = sum(r["out"].astype(np.float64).sum() for r in res.results)
    return np.float32(total)
